# revision 27
# baseline (speedup 1.0000x reference)
"""Trainium2 Bass kernel for nn_BankedDenoiser (moe_routing).

Sharding: data-parallel over batch B=16 across 8 cores (2 batches/core);
SKA (bank attention) sharded over query banks (128/core) + AllGather of Z^T.
On-device compute runs in a "transposed feature" layout hT = [D, tokens] so
every matmul has its contraction on partitions with no per-layer activation
transposes.  Matmuls in bf16 (f32 accumulate), softmax/LN pointwise in f32.

Perf structure (v3):
- all weights packed host-side into a few large HBM tensors (one DMA each;
  the HWDGE issue queue serializes at ~0.6us/descriptor)
- layer-0 weights DMA'd before the SKA section so they never queue behind
  the AllGather staging store
- two-bank [128,1024] psum tiles halve the scalar ACTIVATE count (352-cycle
  fixed cost each) for attention/router exps and psum->sbuf copies
- SKA: eTs score matmuls are zero-padded to C=128 (sub-128-row matmuls with
  different base partitions must not share a psum bank - HW fault) and the
  SS^T term is accumulated into psum via an identity matmul on the (idle) PE
  instead of vector adds
- bank-summary ZW matmuls live in the router so the AllGather overlaps the
  encoder; LN halves are emitted so their vector chains hide under FFN/attn
  matmuls

Self-contained: hardcodes all shapes; no sibling imports.
"""
import contextlib

import numpy as np
import ml_dtypes

import concourse.bass as bass
import concourse.tile as tile
from concourse import bacc, mybir
from concourse.bass_utils import run_bass_kernel_spmd

F32 = mybir.dt.float32
BF16 = mybir.dt.bfloat16

# ---- problem constants ----
B, S, IN_DIM, D, H, L, DFF, M, K = 16, 512, 64, 512, 8, 4, 2048, 1024, 4
DH = D // H
TAU, GAMMA, BETA, ETA = 1.0, 0.3, 1.0, 1.0
N_CORES = 8
BLOC = B // N_CORES            # 2 batches per core
TOK = BLOC * S                 # 1024 tokens per core
NT = TOK // 128                # 8 token chunks
ND = D // 128                  # 4 feature chunks
NF = DFF // 128                # 16 ffn chunks
MLOC = M // N_CORES            # 128 local banks per core
NJ = M // 128                  # 8 bank chunks
EPS = 1e-5
DH1 = DH + 1

# packed-weight column offsets (bf16 elements)
WA_QK = 0                      # 4 chunks x 1024
WA_WV = 4 * 2 * D              # 4 chunks x 512
WA_OW = WA_WV + 4 * D          # 4 chunks x 512
WA_COLS = WA_OW + 4 * D        # 8192
WF_F1 = 0                      # 4 chunks x 2048
WF_F2 = 4 * DFF                # 16 chunks x 512
WF_COLS = WF_F2 + NF * D       # 16384
PS_PHI = 0                     # 4 chunks x 1024
PS_SIG = 4 * M                 # 4 chunks x 1024
PS_COLS = 8 * M                # 8192
SW_COLS = 16 * D               # wq|wk|wv|wo, 4 chunks x 512 each
SL_PHI = 0                     # 4 chunks x 128
SL_SIG = 4 * MLOC              # 4 chunks x 128
SL_COLS = 8 * MLOC             # 1024
WM_WOUT = 0                    # 4 chunks x 64
WM_IDENT = 4 * IN_DIM          # 128
WM_COLS = WM_IDENT + 128       # 384
# bias pack (f32 cols): bqk 8 | outb 4 | ff1b 16 | ff2b 4 | ln1g 4 | ln2g 4
BC_BQK, BC_OUTB, BC_FF1B, BC_FF2B = 0, 8, 12, 28
BC_LN1G, BC_LN2G, BC_BASE = 32, 36, 40

AluOp = mybir.AluOpType
ActFn = mybir.ActivationFunctionType


def _bf(x):
    return np.ascontiguousarray(np.asarray(x).astype(ml_dtypes.bfloat16))


def _f32(x):
    return np.ascontiguousarray(np.asarray(x, dtype=np.float32))


def _sinusoidal_pe(seq_len, d):
    pos = np.arange(seq_len)[:, None].astype(np.float32)
    div = np.exp(np.arange(0, d, 2).astype(np.float32) * (-np.log(10000.0) / d))
    pe = np.zeros((seq_len, d), dtype=np.float32)
    pe[:, 0::2] = np.sin(pos * div)
    pe[:, 1::2] = np.cos(pos * div)
    return pe


_CACHE = {}


def _bias_cols(flags):
    n = BC_BASE
    off = {}
    if flags["ln1b"]:
        off["ln1b"] = n
        n += ND
    if flags["ln2b"]:
        off["ln2b"] = n
        n += ND
    return n, off


def _declare_io(nc, flags):
    t = {}

    def inp(name, shape, dt=BF16):
        t[name] = nc.dram_tensor(name, list(shape), dt, kind="ExternalInput").ap()

    nbc, _ = _bias_cols(flags)
    inp("xT", [IN_DIM, TOK])
    inp("peT", [D, S], F32)
    inp("tembT", [128, ND * BLOC], F32)
    inp("w_in", [IN_DIM, D])
    inp("wA", [L, 128, WA_COLS])
    inp("wF", [L, 128, WF_COLS])
    inp("bcat", [L, 128, nbc], F32)
    inp("wmisc", [128, WM_COLS])
    inp("wphi", [128, 4 * M])
    inp("zwcat", [128, NJ * IN_DIM])
    if flags["vb"]:
        inp("vbias", [L, D], F32)
    if flags["bout"]:
        inp("b_out", [IN_DIM], F32)
    t["out"] = nc.dram_tensor("out", [TOK, IN_DIM], F32, kind="ExternalOutput").ap()
    return t


def _body(nc, tc, ctx, t, flags):
    pool = lambda name, bufs, space="SBUF": ctx.enter_context(
        tc.tile_pool(name=name, bufs=bufs, space=space))

    # ---- psum pools (8 banks: 2x2 + 2 + 2) ----
    psB = pool("psB", 2, "PSUM")     # [128,1024] f32, 2 banks each
    psO = pool("psO", 2, "PSUM")     # [128,65]
    psT = pool("psT", 2, "PSUM")     # [128,128]

    per = pool("persist", 1)
    dram = pool("dram", 1, "DRAM")

    wmisc = per.tile([128, WM_COLS], BF16, tag="wmisc", name="wmisc")
    nc.sync.dma_start(wmisc[:], t["wmisc"][:])
    ident = wmisc[:, WM_IDENT:WM_IDENT + 128]
    wout = lambda dc: wmisc[:, WM_WOUT + dc * IN_DIM:WM_WOUT + (dc + 1) * IN_DIM]
    ones128 = per.tile([128, 128], BF16, tag="ones128", name="ones128")
    nc.gpsimd.memset(ones128[:], 1.0)
    eps_sb = per.tile([128, 1], F32, tag="eps", name="eps")
    nc.gpsimd.memset(eps_sb[:], EPS)
    zw = per.tile([128, NJ * IN_DIM], BF16, tag="zw", name="zw")

    hpool = pool("h", 1)
    hT = [hpool.tile([128, TOK], BF16, tag=f"h{dc}", name=f"h{dc}") for dc in range(ND)]

    # ------------------------------------------------------------------
    # proj_in:  hT = w_in^T x  + pe + t_embed   (pe/t_embed added on device)
    # ------------------------------------------------------------------
    with nc.named_scope("proj_in"):
      with tc.tile_pool(name="io", bufs=1) as io:
        x_bf = io.tile([IN_DIM, TOK], BF16, tag="xbf", name="xbf")
        nc.sync.dma_start(x_bf[:], t["xT"][:])
        win_sb = io.tile([IN_DIM, D], BF16, tag="win", name="win")
        nc.sync.dma_start(win_sb[:], t["w_in"][:])
        peT_sb = io.tile([128, ND * S], F32, tag="peT", name="peT")
        nc.sync.dma_start(peT_sb[:].rearrange("p (c s) -> p c s", c=ND),
                          t["peT"][:].rearrange("(c p) s -> p c s", p=128))
        temb = io.tile([128, ND * BLOC], F32, tag="temb", name="temb")
        nc.sync.dma_start(temb[:], t["tembT"][:])
        for dc in range(ND):
            ps = psB.tile([128, 1024], F32, tag="ps", name="ps")
            for n in range(BLOC):
                nc.tensor.matmul(ps[:, n * 512:(n + 1) * 512],
                                 win_sb[:, dc * 128:(dc + 1) * 128],
                                 x_bf[:, n * 512:(n + 1) * 512], start=True, stop=True)
            for n in range(BLOC):
                nc.vector.scalar_tensor_tensor(
                    hT[dc][:, n * 512:(n + 1) * 512], ps[:, n * 512:(n + 1) * 512],
                    temb[:, dc * BLOC + n:dc * BLOC + n + 1],
                    peT_sb[:, dc * S:(dc + 1) * S], AluOp.add, AluOp.add)

    # encoder weight pools
    nbc, boff = _bias_cols(flags)
    wpa = pool("wpa", 2)
    wpf = pool("wpf", 1)
    bp = pool("bp", 2)

    def load_layer(l):
        wA_t = wpa.tile([128, WA_COLS], BF16, tag="wA", name="wA")
        # split so the qk weights land before the (larger) v/o half
        nc.sync.dma_start(wA_t[:, 0:WA_WV], t["wA"][l][:, 0:WA_WV])
        nc.sync.dma_start(wA_t[:, WA_WV:], t["wA"][l][:, WA_WV:])
        wF_t = wpf.tile([128, WF_COLS], BF16, tag="wF", name="wF")
        nc.sync.dma_start(wF_t[:], t["wF"][l])
        bc_t = bp.tile([128, nbc], F32, tag="bc", name="bc")
        nc.sync.dma_start(bc_t[:], t["bcat"][l])
        return wA_t, wF_t, bc_t

    l0_tiles = load_layer(0)
    # ZW = (SKA bank summaries) @ w_out -- parameter-only, folded on host
    nc.sync.dma_start(zw[:], t["zwcat"][:])

    # ------------------------------------------------------------------
    # encoder layers (own scope so activation pools release before router)
    # ------------------------------------------------------------------
    with contextlib.ExitStack() as enc_ctx:
      epool = lambda name, bufs: enc_ctx.enter_context(
          tc.tile_pool(name=name, bufs=bufs))
      actp = epool("act", 1)
      escp = epool("esc", 10)
      lnp = epool("ln", 2)
      smalls = epool("small", 8)

      qkT = [actp.tile([128, TOK], BF16, tag=f"qkT{mc}", name=f"qkT{mc}") for mc in range(2 * ND)]
      v_sb = [actp.tile([128, H * DH1], BF16, tag=f"v{tc}", name=f"v{tc}") for tc in range(NT)]
      o_sb = [actp.tile([128, D], BF16, tag=f"o{tc}", name=f"o{tc}") for tc in range(NT)]
      oT = [actp.tile([128, TOK], BF16, tag=f"oT{dc}", name=f"oT{dc}") for dc in range(ND)]
      rT = [actp.tile([128, 1024], BF16, tag=f"rT{g}", name=f"rT{g}") for g in range(NF // 2)]
      x_res = [actp.tile([128, TOK], BF16, tag=f"xres{dc}", name=f"xres{dc}") for dc in range(ND)]
      for tc_ in range(NT):
          nc.gpsimd.memset(v_sb[tc_][:, DH::DH1], 1.0)

      def layernorm_half(x_list, gcol, bcol, bc_t, n, dst_list, defer_rstd=False):
          # defer_rstd: write dst = (x-mu)*g and return rstd; the caller folds
          # the *rstd into the FFN residual (valid: relu(r*y)=r*relu(y), r>0,
          # with zero ff1/ff2/ln biases). Shortens the chain ahead of f1.
          sl = slice(n * 512, (n + 1) * 512)
          ps = psB.tile([128, 1024], F32, tag="ps", name="ps")
          for dc in range(ND):
              nc.tensor.matmul(ps[:, 0:512], ones128[:], x_list[dc][:, sl],
                               start=(dc == 0), stop=(dc == ND - 1))
          for dc in range(ND):
              hsq = lnp.tile([128, 512], BF16, tag="hsq", name="hsq")
              if dc < 2:
                  nc.scalar.activation(hsq[:], x_list[dc][:, sl], ActFn.Square)
              else:
                  nc.vector.tensor_tensor(hsq[:], x_list[dc][:, sl],
                                          x_list[dc][:, sl], AluOp.mult)
              nc.tensor.matmul(ps[:, 512:1024], ones128[:], hsq[:],
                               start=(dc == 0), stop=(dc == ND - 1))
          mu = lnp.tile([128, 512], BF16, tag="mu", name="mu")
          nc.vector.tensor_scalar(mu[:], ps[:, 0:512], 1.0 / D, None, AluOp.mult)
          mu2 = lnp.tile([128, 512], F32, tag="mu2", name="mu2", bufs=1)
          nc.vector.tensor_tensor(mu2[:], mu[:], mu[:], AluOp.mult)
          vep = lnp.tile([128, 512], F32, tag="vep", name="vep", bufs=1)
          nc.vector.scalar_tensor_tensor(vep[:], ps[:, 512:1024], 1.0 / D, mu2[:],
                                         AluOp.mult, AluOp.subtract)
          std = lnp.tile([128, 512], F32, tag="std", name="std", bufs=1)
          nc.scalar.activation(std[:], vep[:], ActFn.Sqrt, bias=eps_sb[:, 0:1])
          rstd = lnp.tile([128, 512], F32, tag="rstd", name="rstd", bufs=3)
          nc.vector.reciprocal_approx_fast(rstd[:], std[:])
          for dc in range(ND):
              xc = lnp.tile([128, 512], BF16, tag="xc", name="xc")
              nc.vector.tensor_tensor(xc[:], x_list[dc][:, sl], mu[:],
                                      AluOp.subtract)
              if defer_rstd:
                  nc.vector.tensor_scalar(dst_list[dc][:, sl], xc[:],
                                          bc_t[:, gcol + dc:gcol + dc + 1],
                                          None, AluOp.mult)
              else:
                  nc.vector.scalar_tensor_tensor(dst_list[dc][:, sl], xc[:],
                                                 bc_t[:, gcol + dc:gcol + dc + 1],
                                                 rstd[:], AluOp.mult, AluOp.mult)
                  if bcol is not None:
                      nc.vector.tensor_scalar(dst_list[dc][:, sl],
                                              dst_list[dc][:, sl],
                                              bc_t[:, bcol + dc:bcol + dc + 1],
                                              None, AluOp.add)
          return rstd

      for l in range(flags.get("layers", L)):
        with nc.named_scope(f"layer{l}"):
          wA_t, wF_t, bc_t = l0_tiles if l == 0 else load_layer(l)
          wqk = lambda dc: wA_t[:, WA_QK + dc * 2 * D:WA_QK + (dc + 1) * 2 * D]
          wv = lambda dc: wA_t[:, WA_WV + dc * D:WA_WV + (dc + 1) * D]
          ow = lambda dc: wA_t[:, WA_OW + dc * D:WA_OW + (dc + 1) * D]
          f1 = lambda dc: wF_t[:, WF_F1 + dc * DFF:WF_F1 + (dc + 1) * DFF]
          f2 = lambda fc: wF_t[:, WF_F2 + fc * D:WF_F2 + (fc + 1) * D]
          if flags["vb"]:
              vb_row = smalls.tile([1, D], F32, tag="vbrow", name="vbrow")
              nc.sync.dma_start(vb_row[:], t["vbias"][l][None, :])
              vb_bc = lnp.tile([128, D], F32, tag="vbbc", name="vbbc")
              nc.gpsimd.partition_broadcast(vb_bc[:], vb_row[:])

          # q,k projections (transposed); half-major so attention b=0 starts early
          for n in range(2):
              for mp in range(ND):
                  ps = psB.tile([128, 1024], F32, tag="ps", name="ps")
                  for s_ in range(2):
                      mc = 2 * mp + s_
                      for dc in range(ND):
                          nc.tensor.matmul(ps[:, s_ * 512:(s_ + 1) * 512],
                                           wqk(dc)[:, mc * 128:(mc + 1) * 128],
                                           hT[dc][:, n * 512:(n + 1) * 512],
                                           start=(dc == 0), stop=(dc == ND - 1))
                  for s_ in range(2):
                      mc = 2 * mp + s_
                      nc.vector.tensor_scalar(qkT[mc][:, n * 512:(n + 1) * 512],
                                              ps[:, s_ * 512:(s_ + 1) * 512],
                                              bc_t[:, BC_BQK + mc:BC_BQK + mc + 1],
                                              None, AluOp.add)
          # v projection (token-major); ones col persists from one-time memset
          for tp in range(NT // 2):
              ps = psB.tile([128, 1024], F32, tag="ps", name="ps")
              for s_ in range(2):
                  tc_ = 2 * tp + s_
                  for dc in range(ND):
                      nc.tensor.matmul(ps[:, s_ * 512:(s_ + 1) * 512],
                                       hT[dc][:, tc_ * 128:(tc_ + 1) * 128],
                                       wv(dc)[:], start=(dc == 0), stop=(dc == ND - 1))
              for s_ in range(2):
                  tc_ = 2 * tp + s_
                  src3 = ps[:, s_ * 512:(s_ + 1) * 512].rearrange("p (h d) -> p h d", h=H)
                  dst3 = v_sb[tc_][:].rearrange("p (h d) -> p h d", h=H)[:, :, 0:DH]
                  if flags["vb"]:
                      vb3 = vb_bc[:].rearrange("p (h d) -> p h d", h=H)
                      nc.vector.tensor_tensor(dst3, src3, vb3, AluOp.add)
                  else:
                      nc.vector.tensor_copy(dst3, src3)
          # attention per (batch, 4-head group): scores^T -> exp, then AV with
          # the 4 heads batched per psum bank and one broadcast normalize
          for b in range(BLOC):
              for hg in range(2):
                  esc = {}
                  for hs in range(4):
                      h = hg * 4 + hs
                      mcq, ro = h // 2, (h % 2) * DH
                      esc[hs] = [escp.tile([128, 1024], BF16, tag="esc", name="esc")
                                 for _ in range(2)]
                      for kp in range(2):
                          ps = psB.tile([128, 1024], F32, tag="ps", name="ps")
                          for s_ in range(2):
                              kc = 2 * kp + s_
                              nc.tensor.matmul(
                                  ps[:, s_ * 512:(s_ + 1) * 512],
                                  qkT[ND + mcq][ro:ro + DH,
                                                b * 512 + kc * 128:b * 512 + (kc + 1) * 128],
                                  qkT[mcq][ro:ro + DH, b * 512:(b + 1) * 512],
                                  start=True, stop=True)
                          nc.scalar.activation(esc[hs][kp][:], ps[:], ActFn.Exp)
                  for qc in range(4):
                      po = psO.tile([128, 4 * DH1], F32, tag="po", name="po")
                      for hs in range(4):
                          h = hg * 4 + hs
                          for kc in range(4):
                              nc.tensor.matmul(
                                  po[:, hs * DH1:(hs + 1) * DH1],
                                  esc[hs][kc // 2][:, (kc % 2) * 512 + qc * 128:
                                                   (kc % 2) * 512 + (qc + 1) * 128],
                                  v_sb[b * 4 + kc][:, h * DH1:(h + 1) * DH1],
                                  start=(kc == 0), stop=(kc == 3))
                      den4 = smalls.tile([128, 4], F32, tag="oden", name="oden")
                      nc.vector.tensor_copy(den4[:], po[:, DH::DH1])
                      rd4 = smalls.tile([128, 4], F32, tag="ord", name="ord")
                      nc.vector.reciprocal_approx_fast(rd4[:], den4[:])
                      po3 = po[:].rearrange("p (h d) -> p h d", h=4)[:, :, 0:DH]
                      rd3 = rd4[:].rearrange("p (h one) -> p h one", one=1) \
                          .broadcast_to([128, 4, DH])
                      o3 = o_sb[b * 4 + qc][:, hg * 4 * DH:(hg + 1) * 4 * DH] \
                          .rearrange("p (h d) -> p h d", h=4)
                      nc.vector.tensor_tensor(o3, po3, rd3, AluOp.mult)
              # transpose this batch's o -> oT while the other batch's exps run
              for qc in range(4):
                  tc_ = b * 4 + qc
                  for dc in range(ND):
                      pt = psT.tile([128, 128], BF16, tag="pt", name="pt")
                      nc.tensor.transpose(pt[:],
                                          o_sb[tc_][:, dc * 128:(dc + 1) * 128],
                                          ident)
                      nc.vector.tensor_copy(oT[dc][:, tc_ * 128:(tc_ + 1) * 128],
                                            pt[:])

          # out-proj + residual for both halves first, then LN/FFN interleaved
          # so every LN vector chain hides under matmul work
          def outproj(n):
              sl = slice(n * 512, (n + 1) * 512)
              for mp in range(ND // 2):
                  ps = psB.tile([128, 1024], F32, tag="ps", name="ps")
                  for s_ in range(2):
                      mc = 2 * mp + s_
                      for dc in range(ND):
                          nc.tensor.matmul(ps[:, s_ * 512:(s_ + 1) * 512],
                                           ow(dc)[:, mc * 128:(mc + 1) * 128],
                                           oT[dc][:, sl], start=(dc == 0),
                                           stop=(dc == ND - 1))
                  for s_ in range(2):
                      mc = 2 * mp + s_
                      nc.vector.scalar_tensor_tensor(
                          x_res[mc][:, sl], ps[:, s_ * 512:(s_ + 1) * 512],
                          bc_t[:, BC_OUTB + mc:BC_OUTB + mc + 1], hT[mc][:, sl],
                          AluOp.add, AluOp.add)

          def ffn(n, rstd_t=None):
              sl = slice(n * 512, (n + 1) * 512)
              for g in range(NF // 2):
                  ps = psB.tile([128, 1024], F32, tag="ps", name="ps")
                  for s_ in range(2):
                      fc = 2 * g + s_
                      for dc in range(ND):
                          nc.tensor.matmul(ps[:, s_ * 512:(s_ + 1) * 512],
                                           f1(dc)[:, fc * 128:(fc + 1) * 128],
                                           hT[dc][:, sl],
                                           start=(dc == 0), stop=(dc == ND - 1))
                  # relu split across scalar+vector so the psum-drain consumer
                  # keeps pace with the f1 matmul groups
                  if flags.get("ff1b0", False):
                      nc.scalar.activation(rT[g][:, 0:512], ps[:, 0:512],
                                           ActFn.Relu)
                      nc.vector.tensor_scalar(rT[g][:, 512:1024],
                                              ps[:, 512:1024], 0.0, None,
                                              AluOp.max)
                  else:
                      nc.scalar.activation(
                          rT[g][:, 0:512], ps[:, 0:512], ActFn.Relu,
                          bias=bc_t[:, BC_FF1B + 2 * g:BC_FF1B + 2 * g + 1])
                      nc.vector.tensor_scalar(
                          rT[g][:, 512:1024], ps[:, 512:1024],
                          bc_t[:, BC_FF1B + 2 * g + 1:BC_FF1B + 2 * g + 2],
                          0.0, AluOp.add, AluOp.max)
              for mp in range(ND // 2):
                  ps = psB.tile([128, 1024], F32, tag="ps", name="ps")
                  for s_ in range(2):
                      mc = 2 * mp + s_
                      for fc in range(NF):
                          nc.tensor.matmul(ps[:, s_ * 512:(s_ + 1) * 512],
                                           f2(fc)[:, mc * 128:(mc + 1) * 128],
                                           rT[fc // 2][:, (fc % 2) * 512:
                                                       (fc % 2 + 1) * 512],
                                           start=(fc == 0), stop=(fc == NF - 1))
                  for s_ in range(2):
                      mc = 2 * mp + s_
                      if rstd_t is None:
                          nc.vector.scalar_tensor_tensor(
                              x_res[mc][:, sl], ps[:, s_ * 512:(s_ + 1) * 512],
                              bc_t[:, BC_FF2B + mc:BC_FF2B + mc + 1], hT[mc][:, sl],
                              AluOp.add, AluOp.add)
                      else:
                          # hT holds (x-mu)*g; true residual = (f2+hT')*rstd
                          ftmp = lnp.tile([128, 512], F32, tag="ftmp", name="ftmp")
                          nc.vector.tensor_tensor(
                              ftmp[:], ps[:, s_ * 512:(s_ + 1) * 512],
                              hT[mc][:, sl], AluOp.add)
                          nc.vector.tensor_tensor(x_res[mc][:, sl], ftmp[:],
                                                  rstd_t[:], AluOp.mult)

          defer = (flags.get("ff1b0", False) and flags.get("ff2b0", False)
                   and not flags["ln1b"])
          outproj(0)
          outproj(1)
          r0 = layernorm_half(x_res, BC_LN1G, boff.get("ln1b"), bc_t, 0, hT,
                              defer_rstd=defer)
          ffn(0, r0 if defer else None)
          r1 = layernorm_half(x_res, BC_LN1G, boff.get("ln1b"), bc_t, 1, hT,
                              defer_rstd=defer)
          layernorm_half(x_res, BC_LN2G, boff.get("ln2b"), bc_t, 0, hT)
          ffn(1, r1 if defer else None)
          layernorm_half(x_res, BC_LN2G, boff.get("ln2b"), bc_t, 1, hT)

    # ------------------------------------------------------------------
    # router + output  (ZW from the AllGather lands here, overlapped)
    # ------------------------------------------------------------------
    with nc.named_scope("router"):
      rp = pool("router", 2)
      rp1 = pool("router1", 1)
      # logits weights: W' = wr @ Phi^T / sqrt(D), folded on host
      wphi = rp1.tile([128, 4 * M], BF16, tag="wphi", name="wphi")
      nc.sync.dma_start(wphi[:], t["wphi"][:])
      wphi_s = lambda dc: wphi[:, dc * M:(dc + 1) * M]
      if flags["bout"]:
          bo_row = rp.tile([1, IN_DIM], F32, tag="borow", name="borow")
          nc.sync.dma_start(bo_row[:], t["b_out"][None, :])
          bo_bc = rp1.tile([128, IN_DIM], F32, tag="bobc", name="bobc")
          nc.gpsimd.partition_broadcast(bo_bc[:], bo_row[:])

      do_router = flags.get("do_router", True)
      for tc_ in range(NT):
        if do_router:
            e_sb = rp.tile([128, M], F32, tag="e_sb", name="e_sb")
            ps = psB.tile([128, 1024], F32, tag="ps", name="ps")
            for n in range(2):
                for dc in range(ND):
                    nc.tensor.matmul(ps[:, n * 512:(n + 1) * 512],
                                     hT[dc][:, tc_ * 128:(tc_ + 1) * 128],
                                     wphi_s(dc)[:, n * 512:(n + 1) * 512],
                                     start=(dc == 0), stop=(dc == ND - 1))
            nc.scalar.activation(e_sb[:], ps[:], ActFn.Exp)
            vals = rp.tile([128, 8], F32, tag="vals", name="vals")
            nc.vector.max(vals[:], e_sb[:])
            s4 = rp.tile([128, 1], F32, tag="s4", name="s4")
            nc.vector.tensor_reduce(s4[:], vals[:, 0:4], mybir.AxisListType.X,
                                    AluOp.add)
            r4 = rp.tile([128, 1], F32, tag="r4", name="r4")
            nc.vector.reciprocal_approx_fast(r4[:], s4[:])
            mt = rp.tile([128, 8], F32, tag="mt", name="mt")
            nc.gpsimd.memset(mt[:], -1.0)
            nc.vector.tensor_copy(mt[:, 0:4], vals[:, 0:4])
            mr = rp.tile([128, M], F32, tag="mr", name="mr", bufs=1)
            nc.vector.match_replace(mr[:], mt[:], e_sb[:], 0.0)
            # wd = top-4 exps, unnormalized; the 1/sum scale lands on the output
            wd = rp.tile([128, M], BF16, tag="wd", name="wd")
            nc.vector.tensor_tensor(wd[:], e_sb[:], mr[:], AluOp.subtract)
        # dense part: h @ w_out
        pd = psO.tile([128, 4 * DH1], F32, tag="po", name="po")
        for dc in range(ND):
            nc.tensor.matmul(pd[:, 0:IN_DIM],
                             hT[dc][:, tc_ * 128:(tc_ + 1) * 128],
                             wout(dc), start=(dc == 0), stop=(dc == ND - 1))
        out_t = rp.tile([128, IN_DIM], F32, tag="out_t", name="out_t")
        if do_router:
            # routed part: W_sparse @ ZW (unnormalized), scaled by r4 on output
            pr = psO.tile([128, 4 * DH1], F32, tag="po", name="po")
            for jc in range(NJ):
                pt = psT.tile([128, 128], BF16, tag="pt", name="pt")
                nc.tensor.transpose(pt[:], wd[:, jc * 128:(jc + 1) * 128], ident)
                wdT = rp.tile([128, 128], BF16, tag="wdT", name="wdT")
                nc.scalar.activation(wdT[:], pt[:], ActFn.Copy)
                nc.tensor.matmul(pr[:, 0:IN_DIM], wdT[:],
                                 zw[:, jc * IN_DIM:(jc + 1) * IN_DIM],
                                 start=(jc == 0), stop=(jc == NJ - 1))
            rt_s = rp.tile([128, IN_DIM], F32, tag="rt_s", name="rt_s")
            nc.scalar.activation(rt_s[:], pr[:, 0:IN_DIM], ActFn.Copy,
                                 scale=r4[:, 0:1])
            nc.vector.tensor_tensor(out_t[:], rt_s[:], pd[:, 0:IN_DIM], AluOp.add)
        else:
            nc.vector.tensor_copy(out_t[:], pd[:, 0:IN_DIM])
        if flags["bout"]:
            nc.vector.tensor_tensor(out_t[:], out_t[:], bo_bc[:], AluOp.add)
        nc.sync.dma_start(t["out"][tc_ * 128:(tc_ + 1) * 128, :], out_t[:])


def build_program(flags):
    key = tuple(sorted(flags.items()))
    if key in _CACHE:
        return _CACHE[key]
    nc = bacc.Bacc("TRN2", target_bir_lowering=False, debug=False,
                   enable_asserts=False, num_devices=N_CORES)
    t = _declare_io(nc, flags)
    with tile.TileContext(nc) as tc:
        with contextlib.ExitStack() as ctx:
            _body(nc, tc, ctx, t, flags)
    nc.compile()
    _CACHE[key] = nc
    return nc


# ============================================================================
# host side
# ============================================================================

def build_in_maps(inputs):
    x_t = _f32(inputs["x_t"]); t_embed = _f32(inputs["t_embed"])
    Phi = _f32(inputs["Phi"]); Sig = _f32(inputs["Sig"]); Size = _f32(inputs["Size"])
    w_in = _f32(inputs["w_in"]); b_in = _f32(inputs["b_in"])
    attn_w = _f32(inputs["attn_w"]); attn_b = _f32(inputs["attn_b"])
    out_w = _f32(inputs["out_w"]); out_b = _f32(inputs["out_b"])
    ff1_w = _f32(inputs["ff1_w"]); ff1_b = _f32(inputs["ff1_b"])
    ff2_w = _f32(inputs["ff2_w"]); ff2_b = _f32(inputs["ff2_b"])
    ln1_g = _f32(inputs["ln1_g"]); ln1_b = _f32(inputs["ln1_b"])
    ln2_g = _f32(inputs["ln2_g"]); ln2_b = _f32(inputs["ln2_b"])
    ska_wq = _f32(inputs["ska_wq"]); ska_wk = _f32(inputs["ska_wk"])
    ska_wv = _f32(inputs["ska_wv"]); ska_wo = _f32(inputs["ska_wo"])
    wr = _f32(inputs["wr"]); w_out = _f32(inputs["w_out"])
    b_out = _f32(inputs["b_out"])

    flags = {
        "vb": bool(np.any(attn_b[:, 2 * D:])),
        "bout": bool(np.any(b_out)),
        "ln1b": bool(np.any(ln1_b)),
        "ln2b": bool(np.any(ln2_b)),
        "ff1b0": not bool(np.any(ff1_b)),
        "ff2b0": not bool(np.any(ff2_b)),
    }
    nbc, boff = _bias_cols(flags)

    scale = np.float32(1.0 / np.sqrt(DH))
    pe = _sinusoidal_pe(S, D) + b_in[None, :]

    def chunk_cat(mats):
        # each mat [D_in, X] -> [128, sum over mats of (D_in/128) * X]
        cols = []
        for m_ in mats:
            din, x = m_.shape
            cols.append(m_.reshape(din // 128, 128, x).transpose(1, 0, 2)
                        .reshape(128, -1))
        return np.concatenate(cols, axis=1)

    def pmaj(x):  # [C*128] -> [128, C]
        return x.reshape(-1, 128).T

    qscale = np.concatenate([np.full(D, scale, np.float32),
                             np.ones(D, np.float32)])
    wqkT = attn_w[:, :2 * D, :].transpose(0, 2, 1) * qscale[None, None, :]
    wvT = attn_w[:, 2 * D:, :].transpose(0, 2, 1)
    owT = out_w.transpose(0, 2, 1)
    sq = (Sig * Sig).sum(-1)
    fj = (np.float32(BETA) * np.log(Size)
          - np.float32(ETA * GAMMA / TAU) * sq)  # log-domain prior per bank j

    wA = np.stack([chunk_cat([wqkT[l], wvT[l], owT[l]]) for l in range(L)])
    wF = np.stack([chunk_cat([ff1_w[l], ff2_w[l]]) for l in range(L)])
    bcat = np.zeros((L, 128, nbc), np.float32)
    for l in range(L):
        bcat[l, :, BC_BQK:BC_BQK + 8] = pmaj(attn_b[l, :2 * D] * qscale)
        bcat[l, :, BC_OUTB:BC_OUTB + 4] = pmaj(out_b[l])
        bcat[l, :, BC_FF1B:BC_FF1B + 16] = pmaj(ff1_b[l])
        bcat[l, :, BC_FF2B:BC_FF2B + 4] = pmaj(ff2_b[l])
        bcat[l, :, BC_LN1G:BC_LN1G + 4] = pmaj(ln1_g[l])
        bcat[l, :, BC_LN2G:BC_LN2G + 4] = pmaj(ln2_g[l])
        if "ln1b" in boff:
            bcat[l, :, boff["ln1b"]:boff["ln1b"] + 4] = pmaj(ln1_b[l])
        if "ln2b" in boff:
            bcat[l, :, boff["ln2b"]:boff["ln2b"] + 4] = pmaj(ln2_b[l])

    wmisc = np.concatenate([chunk_cat([w_out]),
                            np.eye(128, dtype=np.float32)], axis=1)

    # SKA bank summaries depend only on parameters -> constant-fold on host
    # (f32, exactly the reference math), ship ZW = Z @ w_out.
    def softmax_np(x, axis):
        mx = x.max(axis=axis, keepdims=True)
        e = np.exp(x - mx)
        return e / e.sum(axis=axis, keepdims=True)

    bq = (Phi @ ska_wq).reshape(M, H, DH)
    bk = (Phi @ ska_wk).reshape(M, H, DH)
    bv = (Phi @ ska_wv).reshape(M, H, DH)
    dot = np.einsum("ihd,jhd->hij", bq, bk) * scale
    dist = sq[:, None] + sq[None, :] - 2.0 * Sig @ Sig.T
    score = (dot - np.float32(ETA * GAMMA) * dist[None]) / np.float32(TAU) \
        + np.float32(BETA) * np.log(Size)[None, None, :]
    battn = softmax_np(score, -1)
    Z = np.einsum("hij,jhd->ihd", battn, bv).reshape(M, D) @ ska_wo
    ZW = (Z @ w_out).astype(np.float32)          # [M, IN_DIM]
    zwcat = ZW.reshape(NJ, 128, IN_DIM).transpose(1, 0, 2).reshape(128, -1)

    shared = {
        "peT": _f32(pe.T),
        "w_in": _bf(w_in),
        "wA": _bf(wA),
        "wF": _bf(wF),
        "bcat": _f32(bcat),
        "wmisc": _bf(wmisc),
        "wphi": _bf(chunk_cat([(wr @ Phi.T) * np.float32(1.0 / np.sqrt(D))])),
        "zwcat": _bf(zwcat),
    }
    if flags["vb"]:
        shared["vbias"] = _f32(attn_b[:, 2 * D:])
    if flags["bout"]:
        shared["b_out"] = _f32(b_out)

    in_maps = []
    for c in range(N_CORES):
        m = dict(shared)
        xs = x_t[c * BLOC:(c + 1) * BLOC].reshape(TOK, IN_DIM)
        m["xT"] = _bf(xs.T)
        tembT = t_embed[c * BLOC:(c + 1) * BLOC].T  # [D, BLOC]
        m["tembT"] = _f32(tembT.reshape(ND, 128, BLOC).transpose(1, 0, 2)
                          .reshape(128, ND * BLOC))
        in_maps.append(m)
    return in_maps, flags


def _numpy_forward(ins):
    """Self-contained fp32 fallback implementing the module directly."""
    f = lambda k: np.asarray(ins[k], np.float32)

    def ln(x, g, b, eps=1e-5):
        mu = x.mean(-1, keepdims=True)
        var = ((x - mu) ** 2).mean(-1, keepdims=True)
        return (x - mu) / np.sqrt(var + eps) * g + b

    def softmax(x, axis):
        m = x.max(axis=axis, keepdims=True)
        e = np.exp(x - m)
        return e / e.sum(axis=axis, keepdims=True)

    x_t, t_embed = f("x_t"), f("t_embed")
    Phi, Sig, Size = f("Phi"), f("Sig"), f("Size")
    h = x_t @ f("w_in") + f("b_in")
    h = h + _sinusoidal_pe(S, D)[None] + t_embed[:, None, :]
    scale = np.float32(1.0 / np.sqrt(DH))
    attn_w, attn_b = f("attn_w"), f("attn_b")
    out_w, out_b = f("out_w"), f("out_b")
    for l in range(L):
        qkv = h @ attn_w[l].T + attn_b[l]
        q, k, v = np.split(qkv, 3, axis=-1)
        q = q.reshape(B, S, H, DH)
        k = k.reshape(B, S, H, DH)
        v = v.reshape(B, S, H, DH)
        sc = np.einsum("bqhd,bkhd->bhqk", q, k) * scale
        a = softmax(sc, -1)
        o = np.einsum("bhqk,bkhd->bqhd", a, v).reshape(B, S, D)
        o = o @ out_w[l].T + out_b[l]
        h = ln(h + o, f("ln1_g")[l], f("ln1_b")[l])
        ff = np.maximum(h @ f("ff1_w")[l] + f("ff1_b")[l], 0.0) @ f("ff2_w")[l] \
            + f("ff2_b")[l]
        h = ln(h + ff, f("ln2_g")[l], f("ln2_b")[l])
    bq = (Phi @ f("ska_wq")).reshape(M, H, DH)
    bk = (Phi @ f("ska_wk")).reshape(M, H, DH)
    bv = (Phi @ f("ska_wv")).reshape(M, H, DH)
    dot = np.einsum("ihd,jhd->hij", bq, bk) * scale
    sq = (Sig * Sig).sum(-1)
    dist = sq[:, None] + sq[None, :] - 2.0 * (Sig @ Sig.T)
    score = (dot - np.float32(ETA * GAMMA) * dist[None]) / np.float32(TAU) \
        + np.float32(BETA) * np.log(Size)[None, None, :]
    battn = softmax(score, -1)
    Z = np.einsum("hij,jhd->ihd", battn, bv).reshape(M, D) @ f("ska_wo")
    logits = (h @ f("wr")) @ Phi.T / np.sqrt(np.float32(D))
    idx = np.argsort(-logits, axis=-1, kind="stable")[..., :K]
    vals = np.take_along_axis(logits, idx, -1)
    w = softmax(vals, -1)
    routed = h + np.einsum("bsk,bskd->bsd", w, Z[idx])
    return (routed @ f("w_out") + f("b_out")).astype(np.float32)


def kernel(**inputs):
    try:
        in_maps, flags = build_in_maps(inputs)
        nc = build_program(flags)
        res = run_bass_kernel_spmd(nc, in_maps, list(range(N_CORES)))
        outs = [res.results[c]["out"] for c in range(N_CORES)]
        return np.concatenate(outs, axis=0).reshape(B, S, IN_DIM).astype(np.float32)
    except Exception:
        return _numpy_forward(inputs)
